# revision 3
# baseline (speedup 1.0000x reference)
"""MHA forward (dense transformer block) for TRN2, 8 NeuronCores.

Sharding: core c handles batch b = c // 4 and head-group g = c % 4
(4 heads of 64 dims = 256 hidden columns).  Wq/Wkv are sharded
column-wise, Wo row-wise; each core produces a partial [2048, 1024]
bf16 output which the host sums per batch (+ bo).

Design notes:
  - q is pre-transposed on the HOST and uploaded bf16 as qt[p, kc, s]
    = q[s, kc*128+p]: no on-device transposes or PSUM->SBUF copies.
  - all weights bf16; projection/scores/attn/outproj matmuls bf16.
  - scores S^T = K^T.T @ Q^T with K=64 head pairs at tile_position
    (0,0)/(64,0): the PE runs both heads' matmuls concurrently
    (row-group tiling; ~41 us/rep measured on HW vs serialized).
  - exp on ACT reads ragged 6/6/4 k-block groups (1536-wide calls)
    to amortize the per-activation overhead; probs bf16.
  - attn@V: lhsT = V' (ones column appended, M=65), rhs = probs ->
    O^T plus the softmax denominator in one accumulation chain.
  - normalize: DVE reciprocal of the denominator row + GPSIMD
    partition_broadcast (keeps the PE out of the normalize chain).
  - rolling software pipeline: sx(j+3) scores/exp interleaved with
    av(j) attn chains, deferred pair-1 projections and outproj as PE
    filler between score groups; ptp bufs=8 decouples ACT from PE.
  - For_i loop body is doubled: each half issues the next half's q
    DMA loads first, so loads sit ahead of the partial stores in the
    SP queue (all DMA stays on the SP queue: triggers on the ACT
    HWDGE queue measurably stall the exp stream).
"""

import sys

sys.path.insert(0, "/opt/trn_rl_repo")

import numpy as np
import ml_dtypes

import concourse.bass as bass
from concourse import bacc
import concourse.mybir as mybir
import concourse.tile as tile
from concourse.bass_utils import run_bass_kernel_spmd

F32 = mybir.dt.float32
F32R = mybir.dt.float32r
BF16 = mybir.dt.bfloat16
AF = mybir.ActivationFunctionType

S = 2048          # sequence length per batch
D = 1024          # model dim
DH = 64           # head dim
NH = 4            # heads per core
GH = NH * DH      # 256 hidden cols per core
VA = DH + 1       # V' cols per head (ones column appended)
KC = D // 128     # 8 contraction chunks of 128
ST = S // 128     # 16 sequence blocks of 128
SPAN = 256        # q-span processed per scores/exp/attnV block
NSP = S // SPAN   # 8 spans
QC = 512          # q-columns per DMA chunk / projection chain
NQC = S // QC     # 4 chunks
SCALE = DH ** -0.5

N_CORES = 8


def _build_nc(reps=1):
    nc = bacc.Bacc("TRN2", target_bir_lowering=False)

    qt = nc.declare_dram_parameter("qt", [128, KC, S], BF16, isOutput=False)
    wq = nc.declare_dram_parameter("wq", [D, GH], BF16, isOutput=False)
    wk = nc.declare_dram_parameter("wk", [D, GH], BF16, isOutput=False)
    wv = nc.declare_dram_parameter("wv", [D, NH * VA], BF16, isOutput=False)
    bqk = nc.declare_dram_parameter("bqk", [128, 4], F32, isOutput=False)
    bvb = nc.declare_dram_parameter("bvb", [128, NH * VA], F32, isOutput=False)
    wo = nc.declare_dram_parameter("wo", [GH, D], BF16, isOutput=False)
    out = nc.declare_dram_parameter("partial", [S, D], BF16, isOutput=True)

    with tile.TileContext(nc) as tc:
        with (
            tc.tile_pool(name="wsing", bufs=1) as wsing,
            tc.tile_pool(name="qtp", bufs=8) as qtp,
            tc.tile_pool(name="ptp", bufs=8) as ptp,
            tc.tile_pool(name="qk", bufs=6) as qk,
            tc.tile_pool(name="vp", bufs=16) as vp,
            tc.tile_pool(name="small", bufs=1) as small,
            tc.tile_pool(name="ostage", bufs=2) as ostage,
            tc.tile_pool(name="pmm", bufs=2, space="PSUM") as pmm,
            tc.tile_pool(name="pst", bufs=2, space="PSUM") as pst,
        ):
            # ---- weights / constants (once) ----
            wq_t = wsing.tile([128, KC, GH], BF16)
            nc.sync.dma_start(out=wq_t, in_=wq[:, :].rearrange("(kc p) f -> p kc f", p=128))
            wk_t = wsing.tile([128, KC, GH], BF16)
            nc.sync.dma_start(out=wk_t, in_=wk[:, :].rearrange("(kc p) f -> p kc f", p=128))
            wv_t = wsing.tile([128, KC, NH * VA], BF16)
            nc.sync.dma_start(out=wv_t, in_=wv[:, :].rearrange("(kc p) f -> p kc f", p=128))
            wo_t = wsing.tile([128, 2, D], BF16)
            nc.sync.dma_start(out=wo_t, in_=wo[:, :].rearrange("(c p) f -> p c f", p=128))
            bqk_t = wsing.tile([128, 4], F32)
            nc.sync.dma_start(out=bqk_t, in_=bqk[:, :])
            bvb_t = wsing.tile([128, NH * VA], F32)
            nc.sync.dma_start(out=bvb_t, in_=bvb[:, :])
            ones_f = wsing.tile([1, 64], F32, name="ones_f")
            nc.vector.memset(ones_f, 1.0)
            ones_c = wsing.tile([1, 64], F32R)
            nc.vector.tensor_copy(ones_c, ones_f)

            consts = (qt, out, wq_t, wk_t, wv_t, wo_t, bqk_t, bvb_t, ones_c)
            pools = (qtp, ptp, qk, vp, small, ostage, pmm, pst)

            def load_q():
                qc = []
                for c in range(NQC):
                    q_tile = qtp.tile([128, KC, QC], BF16, tag="qt")
                    nc.sync.dma_start(
                        out=q_tile, in_=qt[:, :, c * QC:(c + 1) * QC])
                    qc.append(q_tile)
                return qc

            def emit_body(qc_cur, more=True):
                # issue the next body's q loads FIRST so they sit ahead of
                # this body's output stores in the SP DMA queue
                qc_next = load_q() if more else None
                _emit_body(nc, consts, pools, qc_cur)
                return qc_next

            import os
            if reps == 1:
                emit_body(load_q(), more=False)
            elif os.environ.get("BASS_UNROLL"):
                qc = load_q()
                for r in range(reps):
                    qc = emit_body(qc, more=(r < reps - 1))
            else:
                # software-pipelined double body: q loads one body ahead;
                # 8 qtp bufs => the slots alias identically every iteration.
                # An odd rep count peels one body before the doubled loop.
                qc0 = load_q()
                if reps % 2 == 1:
                    qc0 = emit_body(qc0)
                with tc.For_i(0, reps // 2, 1):
                    qc1 = emit_body(qc0)
                    qc0 = emit_body(qc1)

    nc.compile()
    return nc


def _emit_body(nc, consts, pools, qc):
    (qt, out, wq_t, wk_t, wv_t, wo_t, bqk_t, bvb_t, ones_c) = consts
    (qtp, ptp, qk, vp, small, ostage, pmm, pst) = pools

    # ---- projections ----
    QT = [None, None]
    KT = [None, None]

    def qk_chain(pair, w_t, bias_col, dst, c):
        ps = pmm.tile([128, QC], F32, tag="pmm")
        for k in range(KC):
            nc.tensor.matmul(
                ps,
                w_t[:, k, pair * 128:(pair + 1) * 128],
                qc[c][:, k, :],
                start=(k == 0),
                stop=(k == KC - 1),
            )
        nc.vector.tensor_scalar_add(
            dst[:, c * QC:(c + 1) * QC],
            ps,
            bqk_t[:, bias_col:bias_col + 1],
        )

    def qk_proj(pair, w_t, bias_col, dst_list):
        dst = qk.tile([128, S], BF16, tag="qk", name=f"qkproj{pair}_{bias_col}")
        dst_list[pair] = dst
        for c in range(NQC):
            qk_chain(pair, w_t, bias_col, dst, c)

    vpr = []

    def v_proj():
        for sb in range(ST):
            ps = pmm.tile([128, QC], F32, tag="pmm")
            c, cs = sb // 4, sb % 4
            for k in range(KC):
                nc.tensor.matmul(
                    ps[:, :NH * VA],
                    qc[c][:, k, cs * 128:(cs + 1) * 128],
                    wv_t[:, k, :],
                    start=(k == 0),
                    stop=(k == KC - 1),
                )
            v_tile = vp.tile([128, NH * VA], BF16, tag="vp", name=f"vpr{sb}")
            nc.vector.tensor_add(v_tile, ps[:, :NH * VA], bvb_t)
            vpr.append(v_tile)

    OT = [qk.tile([128, S], BF16, tag="qk", name=f"OT{c}") for c in range(2)]

    # ---- attention stages (piecewise, for fine-grained interleaving) ----
    def sx_alloc(pair, sp):
        return [ptp.tile([128, ST, SPAN], BF16, tag="pt",
                         name=f"pt{pair}_{sp}_{h}") for h in range(2)]

    SXG = ((0, 6), (6, 6), (12, 4))  # ragged supergroups of k-blocks

    def sx_grp(pair, sp, pt, grp):
        """Scores + exp for one (head-pair, q-span, ragged k-block group)."""
        q0 = sp * SPAN
        kb0, nkb = SXG[grp]
        ps_st = [pst.tile([128, 6, SPAN], F32, tag="st", name=f"st{h}")
                 for h in range(2)]
        for kk in range(nkb):
            kb = kb0 + kk
            for h in range(2):
                nc.tensor.matmul(
                    ps_st[h][:, kk, :],
                    KT[pair][h * 64:(h + 1) * 64, kb * 128:(kb + 1) * 128],
                    QT[pair][h * 64:(h + 1) * 64, q0:q0 + SPAN],
                    start=True,
                    stop=True,
                )
        for h in range(2):
            nc.scalar.activation(
                pt[h][:, kb0:kb0 + nkb, :],
                ps_st[h][:, :nkb, :],
                AF.Exp,
                scale=SCALE,
            )

    def av_h(pair, sp, pt, h):
        """attn@V + normalize for one (head-pair, q-span, head)."""
        hh = pair * 2 + h
        ov = pmm.tile([128, SPAN], F32, tag="pmm")
        for kb in range(ST):
            nc.tensor.matmul(
                ov[0:VA, :],
                vpr[kb][:, hh * VA:(hh + 1) * VA],
                pt[h][:, kb, :],
                start=(kb == 0),
                stop=(kb == ST - 1),
            )
        rd1 = small.tile([1, SPAN], F32, tag="rd")
        nc.vector.reciprocal(rd1, ov[DH:VA, :])
        rdb = small.tile([64, SPAN], F32, tag="rdb")
        nc.gpsimd.partition_broadcast(rdb, rd1, channels=64)
        nc.vector.tensor_mul(
            OT[pair][h * 64:(h + 1) * 64, sp * SPAN:(sp + 1) * SPAN],
            ov[0:DH, :],
            rdb,
        )

    def outproj_sb(sb):
        o_tile = ostage.tile([128, D], BF16, tag="ostage")
        for n in range(2):
            ps = pmm.tile([128, 512], F32, tag="pmm")
            for c in range(2):
                nc.tensor.matmul(
                    ps,
                    OT[c][:, sb * 128:(sb + 1) * 128],
                    wo_t[:, c, n * 512:(n + 1) * 512],
                    start=(c == 0),
                    stop=(c == 1),
                )
            nc.vector.tensor_copy(o_tile[:, n * 512:(n + 1) * 512], ps)
        nc.sync.dma_start(out=out[sb * 128:(sb + 1) * 128, :], in_=o_tile)

    # ---- emission order: software-pipelined attention ----
    # Front: K^T/Q^T pair0 + V' (needed by the first avs); pair1
    # projections are deferred into the rolling loop as PE fillers.
    qk_proj(0, wk_t, 2, KT)
    QT[0] = qk.tile([128, S], BF16, tag="qk", name="qkproj0_0")
    seq = [(0, sp) for sp in range(NSP)] + [(1, sp) for sp in range(NSP)]
    pts = {}

    def sx_full(entry):
        pair, sp = entry
        pt = sx_alloc(pair, sp)
        pts[entry] = pt
        for grp in range(len(SXG)):
            sx_grp(pair, sp, pt, grp)

    qk_chain(0, wq_t, 0, QT[0], 0)
    sx_full(seq[0])
    qk_chain(0, wq_t, 0, QT[0], 1)
    sx_full(seq[1])
    qk_chain(0, wq_t, 0, QT[0], 2)
    sx_full(seq[2])
    qk_chain(0, wq_t, 0, QT[0], 3)
    v_proj()

    KT[1] = qk.tile([128, S], BF16, tag="qk", name="qkproj1_3")
    QT[1] = qk.tile([128, S], BF16, tag="qk", name="qkproj1_1")

    LOOK = 3  # sx lookahead over av; ptp bufs must be >= 2*(LOOK+1)

    # Deferred PE work queue: (min_iter, fn) where iter i runs av(seq[i])
    # and sx(seq[i+LOOK]).  Chains for pair1 must land before their
    # consumers: KT1 complete + QT1 c0 before sx(1,0) at i=8-LOOK;
    # QT1 c before sx(1, 2c) at i=8+2c-LOOK.
    fillers = []
    for c in range(NQC):
        fillers.append((c, lambda c=c: qk_chain(1, wk_t, 3, KT[1], c)))
    for c in range(NQC):
        fillers.append((max(4, 8 + 2 * (c - 1) - LOOK),
                        lambda c=c: qk_chain(1, wq_t, 1, QT[1], c)))

    done_out = 0
    for i in range(len(seq)):
        entry_new = seq[i + LOOK] if i + LOOK < len(seq) else None
        pair_o, sp_o = seq[i]
        pt_o = pts.pop(seq[i])

        # build this iteration's filler list (PE work not gated on exp)
        fill = []
        fill.append(lambda: av_h(pair_o, sp_o, pt_o, 0))
        fill.append(lambda: av_h(pair_o, sp_o, pt_o, 1))
        while fillers and fillers[0][0] <= i:
            fill.append(fillers.pop(0)[1])
        if pair_o == 1 and sp_o % 2 == 1:
            spair = sp_o // 2
            for sl in range(4):
                fill.append(lambda sb=spair * 4 + sl: outproj_sb(sb))
            done_out += 1

        if entry_new is not None:
            pt = sx_alloc(*entry_new)
            pts[entry_new] = pt
            ngrp = len(SXG)
            for grp in range(ngrp):
                sx_grp(entry_new[0], entry_new[1], pt, grp)
                # interleave ~equal filler work after each score group
                take = (len(fill) + ngrp - 1 - grp) // (ngrp - grp)
                for fn in fill[:take]:
                    fn()
                fill = fill[take:]
        for fn in fill:
            fn()
    assert done_out == 4


_NC_CACHE = {}


def _get_nc(reps=1):
    if reps not in _NC_CACHE:
        _NC_CACHE[reps] = _build_nc(reps)
    return _NC_CACHE[reps]


def _shard_inputs(q, Wq, bq, Wkv, bkv, Wo, bo):
    q = np.asarray(q, dtype=np.float32)
    Wq = np.asarray(Wq, dtype=np.float32)
    bq = np.asarray(bq, dtype=np.float32)
    Wkv = np.asarray(Wkv, dtype=np.float32)
    bkv = np.asarray(bkv, dtype=np.float32)
    Wo = np.asarray(Wo, dtype=np.float32)

    HID = D  # 1024 total hidden
    in_maps = []
    for c in range(N_CORES):
        b, g = divmod(c, 4)
        lo = g * GH
        wk_s = Wkv[:, lo:lo + GH]
        wv_s = Wkv[:, HID + lo:HID + lo + GH]
        bq_s = bq[lo:lo + GH]
        bk_s = bkv[lo:lo + GH]
        bv_s = bkv[HID + lo:HID + lo + GH]

        wv_aug = np.zeros((D, NH * VA), dtype=np.float32)
        bv_aug = np.zeros((NH * VA,), dtype=np.float32)
        for h in range(NH):
            wv_aug[:, h * VA:h * VA + DH] = wv_s[:, h * DH:(h + 1) * DH]
            bv_aug[h * VA:h * VA + DH] = bv_s[h * DH:(h + 1) * DH]
            bv_aug[h * VA + DH] = 1.0

        bqk_pp = np.stack(
            [bq_s[0:128], bq_s[128:256], bk_s[0:128], bk_s[128:256]], axis=1
        )

        # qt[p, kc, s] = q[b][s, kc*128+p], bf16
        qt = np.ascontiguousarray(
            q[b].T.reshape(KC, 128, S).transpose(1, 0, 2)
        ).astype(ml_dtypes.bfloat16)

        in_maps.append({
            "qt": qt,
            "wq": np.ascontiguousarray(Wq[:, lo:lo + GH]).astype(ml_dtypes.bfloat16),
            "wk": np.ascontiguousarray(wk_s).astype(ml_dtypes.bfloat16),
            "wv": wv_aug.astype(ml_dtypes.bfloat16),
            "bqk": np.ascontiguousarray(bqk_pp),
            "bvb": np.broadcast_to(bv_aug, (128, NH * VA)).copy(),
            "wo": np.ascontiguousarray(Wo[lo:lo + GH, :]).astype(ml_dtypes.bfloat16),
        })
    return in_maps


def _gather(results, bo):
    bo = np.asarray(bo, dtype=np.float32)
    out = np.empty((2, S, D), dtype=np.float32)
    for b in range(2):
        acc = results[4 * b]["partial"].astype(np.float32)
        for g in range(1, 4):
            acc = acc + results[4 * b + g]["partial"]
        out[b] = acc + bo
    return out


_RUNNER_CACHE = {}


def _make_runner(reps=1):
    """Build (once) a reusable jitted SPMD callable for the given rep count.

    Re-jitting per call loads a second copy of the NEFF and has been seen to
    wedge the exec unit, so the jitted executable is cached per process.
    """
    if reps in _RUNNER_CACHE:
        return _RUNNER_CACHE[reps]

    import jax
    from jax.sharding import Mesh, PartitionSpec
    from jax.experimental.shard_map import shard_map
    from concourse import bass2jax

    nc = _get_nc(reps)
    bass2jax.install_neuronx_cc_hook()
    partition_name = nc.partition_id_tensor.name if nc.partition_id_tensor else None
    in_names, out_names, out_avals, zero_outs = [], [], [], []
    for alloc in nc.m.functions[0].allocations:
        if not isinstance(alloc, mybir.MemoryLocationSet):
            continue
        name = alloc.memorylocations[0].name
        if alloc.kind == "ExternalInput":
            if name != partition_name:
                in_names.append(name)
        elif alloc.kind == "ExternalOutput":
            out_names.append(name)
            shape = tuple(alloc.tensor_shape)
            dtype = mybir.dt.np(alloc.dtype)
            out_avals.append(jax.core.ShapedArray(shape, dtype))
            zero_outs.append(np.zeros(shape, dtype))
    n_params = len(in_names)
    n_outs = len(out_avals)
    in_names.extend(out_names)
    if partition_name:
        in_names.append(partition_name)

    def _body(*args):
        operands = list(args)
        if partition_name:
            operands.append(bass2jax.partition_id_tensor())
        return tuple(bass2jax._bass_exec_p.bind(
            *operands,
            out_avals=tuple(out_avals),
            in_names=tuple(in_names),
            out_names=tuple(out_names),
            lowering_input_output_aliases=(),
            sim_require_finite=True,
            sim_require_nnan=True,
            nc=nc,
        ))

    devices = jax.devices()[:N_CORES]
    mesh = Mesh(np.asarray(devices), ("core",))
    donate = tuple(range(n_params, n_params + n_outs))
    sharded = jax.jit(
        shard_map(_body, mesh=mesh,
                  in_specs=(PartitionSpec("core"),) * (n_params + n_outs),
                  out_specs=(PartitionSpec("core"),) * len(out_names),
                  check_rep=False),
        donate_argnums=donate, keep_unused=True)

    def run(in_maps):
        per_core = [[np.asarray(m[nm]) for nm in in_names[:n_params]]
                    for m in in_maps]
        concat_in = [np.concatenate([per_core[c][i] for c in range(N_CORES)],
                                    axis=0) for i in range(n_params)]
        zo = [np.concatenate([z] * N_CORES, axis=0) for z in zero_outs]
        outs = sharded(*concat_in, *zo)
        outs = [np.asarray(o) for o in outs]
        per_core_res = []
        for c in range(N_CORES):
            per_core_res.append({
                name: np.split(outs[i], N_CORES, axis=0)[c]
                for i, name in enumerate(out_names)
            })
        return per_core_res

    _RUNNER_CACHE[reps] = run
    return run


def _run(inputs, reps=1):
    run = _make_runner(reps)
    in_maps = _shard_inputs(**inputs)
    results = run(in_maps)
    out = _gather(results, inputs["bo"])
    return out, results


def kernel(q, Wq, bq, Wkv, bkv, Wo, bo):
    out, _ = _run(dict(q=q, Wq=Wq, bq=bq, Wkv=Wkv, bkv=bkv, Wo=Wo, bo=bo))
    return out
